# revision 19
# baseline (speedup 1.0000x reference)
"""Trainium2 Bass kernel for nn_ComplexCrossAttention.

Sharding: 8 cores = 2 batches x 4 head-groups (4 heads each).
Each core computes, for its (b, head-group):
  - complex Q/K/V projections via Karatsuba (3 real matmuls instead of 4):
      t1 = xr@wr, t2 = xi@wi, t3 = (xr+xi)@(wr+wi)
      re = t1 - t2, im = t3 - t1 - t2
    Q/K pack two heads per matmul (m=128); the head-pair outputs are
    recombined into per-head (re|im) layout with two small SBUF->SBUF
    partition-shift DMAs per block.
  - attention scoresT = (qr.kr + qi.ki)*scale with s on partitions
  - softmax (no max-subtraction; scores are provably small) via exp + column-sum
  - av in transposed layout -> per-head OT [d2, l], normalized and packed
    into pair-major (or|or) / (oi|oi) tiles
  - output projection via Karatsuba over pair-packed contraction blocks
Host sums the 4 partial y per batch and adds the bias.

Activations/weights are fp16 (full-rate on PE, 4x less rounding error than
bf16); x/ctx (and their Karatsuba sums) are pre-cast to fp16 on the host so
DMA transposes start immediately. expT stays bf16 (exp can exceed fp16 max).
"""

import sys

import numpy as np

try:
    import concourse.bacc as bacc
except ImportError:  # pragma: no cover - fallback for bare environments
    sys.path.insert(0, "/opt/trn_rl_repo")
    import concourse.bacc as bacc

import concourse.mybir as mybir
import concourse.tile as tile
from concourse.bass_utils import run_bass_kernel_spmd

F32 = mybir.dt.float32
BF16 = mybir.dt.bfloat16
FP16 = mybir.dt.float16

# ---- problem constants (hardcoded per contract) ----
B, L, S, C = 2, 2048, 2048, 1024
H, D = 16, 64
SCALE = float(1.0 / np.sqrt(np.float32(D)))
HPC = 4          # heads per core
NPAIR = 2        # head pairs per core
D2 = 2 * D       # stacked (real|imag) head dim = 128
NCK = C // 128   # contraction chunks = 8
NLB = L // 512   # l-blocks = 4
NST = S // 128   # s-tiles = 16
NEB = 2          # e-blocks of 512 in C

ACT_DT = FP16    # activations + weights
EXP_DT = BF16    # expT (range: exp(scores) can exceed fp16 max)

_CACHE = {}


def _build_program():
    nc = bacc.Bacc("TRN2", target_bir_lowering=False, debug=False, num_devices=8)

    # per-core external inputs (host pre-cast to fp16, incl. Karatsuba sums)
    x_r = nc.dram_tensor("x_r", [L, C], ACT_DT, kind="ExternalInput")
    x_i = nc.dram_tensor("x_i", [L, C], ACT_DT, kind="ExternalInput")
    x_s = nc.dram_tensor("x_s", [L, C], ACT_DT, kind="ExternalInput")
    c_r = nc.dram_tensor("c_r", [S, C], ACT_DT, kind="ExternalInput")
    c_i = nc.dram_tensor("c_i", [S, C], ACT_DT, kind="ExternalInput")
    c_s = nc.dram_tensor("c_s", [S, C], ACT_DT, kind="ExternalInput")
    # Karatsuba weight banks (host-prepared):
    # wq/wk: [C, pair, term, 128] lhsT tiles (term: wr | wi | wr+wi; 2 heads packed)
    wq = nc.dram_tensor("wq", [C, NPAIR, 3, D2], ACT_DT, kind="ExternalInput")
    wk = nc.dram_tensor("wk", [C, NPAIR, 3, D2], ACT_DT, kind="ExternalInput")
    # wv: [C, term, HPC*64] rhs tiles
    wv = nc.dram_tensor("wv", [C, 3, HPC * D], ACT_DT, kind="ExternalInput")
    # wo: [pair, term, 128 (d-rows of head a|b), eb, 512] rhs tiles
    wo = nc.dram_tensor("wo", [NPAIR, 3, D2, NEB, 512], ACT_DT, kind="ExternalInput")

    y_r = nc.dram_tensor("y_r", [L, C], FP16, kind="ExternalOutput")
    y_i = nc.dram_tensor("y_i", [L, C], FP16, kind="ExternalOutput")

    import os
    dbg = None
    if os.environ.get("KARA_DEBUG"):
        dbg = {
            "dq": nc.dram_tensor("dq", [128, HPC, L], F32, kind="ExternalOutput"),
            "dk": nc.dram_tensor("dk", [128, HPC, S], F32, kind="ExternalOutput"),
            "dv": nc.dram_tensor("dv", [128, NST, HPC, 2, D], F32, kind="ExternalOutput"),
            "dor": nc.dram_tensor("dor", [128, NPAIR, 512], F32, kind="ExternalOutput"),
            "doi": nc.dram_tensor("doi", [128, NPAIR, 512], F32, kind="ExternalOutput"),
        }

    with tile.TileContext(nc) as tc:
        _emit(nc, tc, x_r, x_i, x_s, c_r, c_i, c_s, wq, wk, wv, wo, y_r, y_i, dbg)

    nc.compile()
    return nc


def _emit(nc, tc, x_r, x_i, x_s, c_r, c_i, c_s, wq, wk, wv, wo, y_r, y_i, dbg=None):
    from contextlib import ExitStack

    # DMA-capable queues for spreading transposes / shifts / stores
    ctx = ExitStack()
    with ctx:
        attn_sb = ctx.enter_context(tc.tile_pool(name="attn_sb", bufs=1))

        # persistent attention operands
        qs = attn_sb.tile([128, HPC, L], ACT_DT)              # [d2, h, l]
        ks = attn_sb.tile([128, HPC, S], ACT_DT)              # [d2, h, s]
        vs = attn_sb.tile([128, NST, HPC, 2, D], ACT_DT)      # [s, st, h, ri, d]

        # ---------- P1: transpose-in x, Q projection (Karatsuba) ----------
        with (
            tc.tile_pool(name="xt", bufs=1) as xt_pool,
            tc.tile_pool(name="wq", bufs=1) as wq_pool,
            tc.tile_pool(name="qsc", bufs=2) as qsc_pool,
            tc.tile_pool(name="qus", bufs=2) as qus_pool,
            tc.tile_pool(name="ps_q", bufs=2, space="PSUM") as ps_q,
        ):
            wq_sb = wq_pool.tile([128, NCK, NPAIR, 3, D2], ACT_DT, tag="wq")
            nc.sync.dma_start(
                out=wq_sb, in_=wq.rearrange("(ck p) pr t m -> p ck pr t m", p=128)
            )
            xt = [xt_pool.tile([128, NCK, L], ACT_DT, tag=f"xt{t}", name=f"xt{t}")
                  for t in range(3)]
            for ck in range(NCK):
                for t, src in enumerate((x_r, x_i, x_s)):
                    nc.sync.dma_start(
                        out=xt[t][:, ck, :],
                        in_=src[:, ck * 128:(ck + 1) * 128],
                        transpose=True,
                    )
            for p in range(NPAIR):
                a, b = 2 * p, 2 * p + 1
                for lb in range(NLB):
                    lsl = slice(lb * 512, (lb + 1) * 512)
                    pt = ps_q.tile([128, 3, 512], F32, tag="pq", name="pq")
                    for ck in range(NCK):
                        for t in range(3):
                            nc.tensor.matmul(
                                pt[:, t, :],
                                wq_sb[:, ck, p, t, :],
                                xt[t][:, ck, lsl],
                                start=(ck == 0),
                                stop=(ck == NCK - 1),
                                skip_group_check=True,
                            )
                    # re = t1-t2, im = t3-t1-t2 ; halves for head a (rows 0:64)
                    # and head b (rows 64:128); misaligned halves go via a
                    # scratch tile + partition-shift DMA. Engines may read at
                    # most one PSUM operand, so t2 and u=t1+t2 stage via SBUF.
                    qsc = qsc_pool.tile([128, 512], ACT_DT, tag="qsc")
                    t2s = qus_pool.tile([128, 512], F32, tag="t2s")
                    us = qus_pool.tile([128, 512], F32, tag="us")
                    nc.scalar.copy(out=t2s, in_=pt[:, 1, :])
                    nc.vector.tensor_sub(
                        out=qs[0:64, a, lsl], in0=pt[0:64, 0, :], in1=t2s[0:64, :])
                    nc.vector.tensor_sub(
                        out=qsc[64:128, :], in0=pt[64:128, 0, :], in1=t2s[64:128, :])
                    nc.vector.tensor_add(
                        out=us, in0=pt[:, 0, :], in1=t2s)
                    nc.vector.tensor_sub(
                        out=qsc[0:64, :], in0=pt[0:64, 2, :], in1=us[0:64, :])
                    nc.vector.tensor_sub(
                        out=qs[64:128, b, lsl], in0=pt[64:128, 2, :], in1=us[64:128, :])
                    nc.sync.dma_start(out=qs[0:64, b, lsl], in_=qsc[64:128, :])
                    nc.scalar.dma_start(out=qs[64:128, a, lsl], in_=qsc[0:64, :])

        # ---------- P3: transpose ctx, K and V projections (Karatsuba) ----
        # score/exp pools open across P3 so the scheduler can hoist
        # scoresT+exp of early heads into K/V-phase gaps.
        exp_pool = ctx.enter_context(tc.tile_pool(name="exp", bufs=1))
        ps_s = ctx.enter_context(tc.tile_pool(name="ps_s", bufs=1, space="PSUM"))

        with (
            tc.tile_pool(name="ct", bufs=1) as ct_pool,
            tc.tile_pool(name="wkv", bufs=1) as wkv_pool,
            tc.tile_pool(name="ksc", bufs=2) as ksc_pool,
            tc.tile_pool(name="kus", bufs=2) as kus_pool,
            tc.tile_pool(name="ps_k", bufs=1, space="PSUM") as ps_k,
            tc.tile_pool(name="ps_v", bufs=1, space="PSUM") as ps_v,
        ):
            wk_sb = wkv_pool.tile([128, NCK, NPAIR, 3, D2], ACT_DT, tag="wk")
            nc.sync.dma_start(
                out=wk_sb, in_=wk.rearrange("(ck p) pr t m -> p ck pr t m", p=128)
            )
            wv_sb = wkv_pool.tile([128, NCK, 3, HPC * D], ACT_DT, tag="wv")
            nc.sync.dma_start(
                out=wv_sb, in_=wv.rearrange("(ck p) t n -> p ck t n", p=128)
            )
            ct = [ct_pool.tile([128, NCK, S], ACT_DT, tag=f"ct{t}", name=f"ct{t}")
                  for t in range(3)]
            for ck in range(NCK):
                for t, src in enumerate((c_r, c_i, c_s)):
                    nc.sync.dma_start(
                        out=ct[t][:, ck, :],
                        in_=src[:, ck * 128:(ck + 1) * 128],
                        transpose=True,
                    )
            for p in range(NPAIR):
                a, b = 2 * p, 2 * p + 1
                for sb in range(S // 512):
                    ssl = slice(sb * 512, (sb + 1) * 512)
                    pk = ps_k.tile([128, 3, 512], F32, tag="pk", name="pk")
                    for ck in range(NCK):
                        for t in range(3):
                            nc.tensor.matmul(
                                pk[:, t, :],
                                wk_sb[:, ck, p, t, :],
                                ct[t][:, ck, ssl],
                                start=(ck == 0),
                                stop=(ck == NCK - 1),
                                skip_group_check=True,
                            )
                    ksc = ksc_pool.tile([128, 512], ACT_DT, tag="ksc")
                    t2k = kus_pool.tile([128, 512], F32, tag="t2k")
                    uk = kus_pool.tile([128, 512], F32, tag="uk")
                    nc.scalar.copy(out=t2k, in_=pk[:, 1, :])
                    nc.vector.tensor_sub(
                        out=ks[0:64, a, ssl], in0=pk[0:64, 0, :], in1=t2k[0:64, :])
                    nc.vector.tensor_sub(
                        out=ksc[64:128, :], in0=pk[64:128, 0, :], in1=t2k[64:128, :])
                    nc.vector.tensor_add(
                        out=uk, in0=pk[:, 0, :], in1=t2k)
                    nc.vector.tensor_sub(
                        out=ksc[0:64, :], in0=pk[0:64, 2, :], in1=uk[0:64, :])
                    nc.vector.tensor_sub(
                        out=ks[64:128, b, ssl], in0=pk[64:128, 2, :], in1=uk[64:128, :])
                    nc.sync.dma_start(out=ks[0:64, b, ssl], in_=ksc[64:128, :])
                    nc.scalar.dma_start(out=ks[64:128, a, ssl], in_=ksc[0:64, :])
            for st in range(NST):
                pv = ps_v.tile([128, 3, 512], F32, tag="pv")
                for ck in range(NCK):
                    for t in range(3):
                        nc.tensor.matmul(
                            pv[:, t, 0:HPC * D],
                            ct[t][:, ck, st * 128:(st + 1) * 128],
                            wv_sb[:, ck, t, :],
                            start=(ck == 0),
                            stop=(ck == NCK - 1),
                            skip_group_check=True,
                        )
                # vr/vi land in per-head (ri, d) slots via strided free-dim APs
                t2v = kus_pool.tile([128, HPC * D], F32, tag="t2v")
                uv = kus_pool.tile([128, HPC * D], F32, tag="uv")
                nc.scalar.copy(out=t2v, in_=pv[:, 1, 0:HPC * D])
                nc.vector.tensor_sub(
                    out=vs[:, st, :, 0, :],
                    in0=pv[:, 0, 0:HPC * D],
                    in1=t2v,
                )
                nc.vector.tensor_add(
                    out=uv, in0=pv[:, 0, 0:HPC * D], in1=t2v)
                nc.vector.tensor_sub(
                    out=vs[:, st, :, 1, :],
                    in0=pv[:, 2, 0:HPC * D],
                    in1=uv,
                )

        if dbg is not None:
            nc.gpsimd.dma_start(out=dbg["dq"][:, :, :], in_=qs)
            nc.gpsimd.dma_start(out=dbg["dk"][:, :, :], in_=ks)
            nc.gpsimd.dma_start(out=dbg["dv"][:, :, :, :, :], in_=vs)

        # ---------- P4+P5 fused: attention + output projection, lb-outer ----------
        with (
            tc.tile_pool(name="late", bufs=1) as late_pool,
            tc.tile_pool(name="exp2", bufs=3) as exp_pool2,
            tc.tile_pool(name="otp", bufs=2) as ot_pool,
            tc.tile_pool(name="osc", bufs=2) as osc_pool,
            tc.tile_pool(name="ysb", bufs=2) as ysb_pool,
            tc.tile_pool(name="ps_d", bufs=1, space="PSUM") as ps_d,
            tc.tile_pool(name="ps_o", bufs=1, space="PSUM") as ps_o,
            tc.tile_pool(name="ps_y", bufs=1, space="PSUM") as ps_y,
        ):
            ones = late_pool.tile([128, D2], EXP_DT)
            nc.vector.memset(ones, 1.0)
            wo_sb = late_pool.tile([128, NPAIR, 3, NEB, 512], ACT_DT, tag="wo")
            nc.sync.dma_start(out=wo_sb, in_=wo.rearrange("pr t p eb e -> p pr t eb e"))
            for lb in range(NLB):
                lsl = slice(lb * 512, (lb + 1) * 512)
                # pair-major packed attention outputs for this l-block
                orp = ot_pool.tile([128, NPAIR, 512], ACT_DT, tag="orp", name="orp")
                oip = ot_pool.tile([128, NPAIR, 512], ACT_DT, tag="oip", name="oip")
                osp = ot_pool.tile([128, NPAIR, 512], ACT_DT, tag="osp", name="osp")
                for h in range(HPC):
                    p, is_b = h // 2, h % 2
                    pool_h = exp_pool if lb == 0 else exp_pool2
                    expt = pool_h.tile([128, NST, 512], EXP_DT, tag="expt", name="expt")
                    for pr in range(NST // 2):
                        pscore = ps_s.tile([128, 2, 512], F32, tag="pscore")
                        for j in range(2):
                            st = 2 * pr + j
                            nc.tensor.matmul(
                                pscore[:, j, :],
                                ks[:, h, st * 128:(st + 1) * 128],
                                qs[:, h, lsl],
                                start=True,
                                stop=True,
                                skip_group_check=True,
                            )
                        nc.scalar.activation(
                            out=expt[:, 2 * pr:2 * pr + 2, :],
                            in_=pscore,
                            func=mybir.ActivationFunctionType.Exp,
                            scale=SCALE,
                        )
                    # av: OT[d2, l] accumulated over s-tiles (reads expt first)
                    pav = ps_o.tile([128, 512], F32, tag="pav")
                    for st in range(NST):
                        nc.tensor.matmul(
                            pav,
                            vs[:, st, h, :, :],
                            expt[:, st, :],
                            start=(st == 0),
                            stop=(st == NST - 1),
                        )
                    # in-place pairwise tree-sum of the 16 s-tiles (WAR after av)
                    for step in (1, 2, 4, 8):
                        eng = nc.gpsimd if step == 1 else nc.vector
                        for j in range(0, NST, 2 * step):
                            eng.tensor_add(
                                out=expt[:, j, :], in0=expt[:, j, :], in1=expt[:, j + step, :]
                            )
                    pden = ps_d.tile([128, 512], F32, tag="pden")
                    nc.tensor.matmul(pden, ones, expt[:, 0, :], start=True, stop=True)
                    recip = ot_pool.tile([128, 512], F32, tag="recip")
                    nc.vector.reciprocal(out=recip, in_=pden)
                    # normalize + pack into pair-major tiles; misaligned halves
                    # go via scratch + partition-shift DMA
                    osc = osc_pool.tile([128, 512], ACT_DT, tag="osc")
                    if is_b == 0:  # head a: or aligned, oi shifts down
                        nc.vector.tensor_mul(
                            out=orp[0:64, p, :], in0=pav[0:64, :], in1=recip[0:64, :])
                        nc.vector.tensor_mul(
                            out=osc[64:128, :], in0=pav[64:128, :], in1=recip[64:128, :])
                        nc.sync.dma_start(out=oip[0:64, p, :], in_=osc[64:128, :])
                    else:          # head b: oi aligned, or shifts up
                        nc.vector.tensor_mul(
                            out=osc[0:64, :], in0=pav[0:64, :], in1=recip[0:64, :])
                        nc.vector.tensor_mul(
                            out=oip[64:128, p, :], in0=pav[64:128, :], in1=recip[64:128, :])
                        nc.scalar.dma_start(out=orp[64:128, p, :], in_=osc[0:64, :])
                nc.gpsimd.tensor_add(out=osp, in0=orp, in1=oip)
                if dbg is not None and lb == 0:
                    nc.gpsimd.dma_start(out=dbg["dor"][:, :, :], in_=orp)
                    nc.gpsimd.dma_start(out=dbg["doi"][:, :, :], in_=oip)

                # output projection for this l-block (Karatsuba over pair blocks)
                for jt in range(4):
                    lt = lb * 4 + jt
                    lrow = slice(lt * 128, (lt + 1) * 128)
                    jsl = slice(jt * 128, (jt + 1) * 128)
                    for eb in range(NEB):
                        esl = slice(eb * 512, (eb + 1) * 512)
                        py = ps_y.tile([128, 3, 512], F32, tag="py", name="py")
                        for t, term in enumerate((orp, oip, osp)):
                            for p in range(NPAIR):
                                nc.tensor.matmul(
                                    py[:, t, :],
                                    term[:, p, jsl],
                                    wo_sb[:, p, t, eb, :],
                                    start=(p == 0),
                                    stop=(p == NPAIR - 1),
                                    skip_group_check=True,
                                )
                        t2o = ysb_pool.tile([128, 512], F32, tag="t2o")
                        uo = ysb_pool.tile([128, 512], F32, tag="uo")
                        nc.scalar.copy(out=t2o, in_=py[:, 1, :])
                        yr_t = ysb_pool.tile([128, 512], FP16, tag="yrt")
                        nc.vector.tensor_sub(out=yr_t, in0=py[:, 0, :], in1=t2o)
                        nc.sync.dma_start(out=y_r[lrow, esl], in_=yr_t)
                        nc.vector.tensor_add(out=uo, in0=py[:, 0, :], in1=t2o)
                        yi_t = ysb_pool.tile([128, 512], FP16, tag="yit")
                        nc.vector.tensor_sub(out=yi_t, in0=py[:, 2, :], in1=uo)
                        nc.scalar.dma_start(out=y_i[lrow, esl], in_=yi_t)


def _prep_core_inputs(inputs, core):
    """Slice + host-prepare activation/weight layouts for one core."""
    import ml_dtypes  # noqa: F401  (fp16 is native numpy)

    b = core // 4
    g = core % 4
    hcols = slice(g * HPC * D, (g + 1) * HPC * D)  # 256 channel cols/rows

    wq_r = inputs["wq_r"][:, hcols].astype(np.float32)
    wq_i = inputs["wq_i"][:, hcols].astype(np.float32)
    wk_r = inputs["wk_r"][:, hcols].astype(np.float32)
    wk_i = inputs["wk_i"][:, hcols].astype(np.float32)
    wv_r = inputs["wv_r"][:, hcols].astype(np.float32)
    wv_i = inputs["wv_i"][:, hcols].astype(np.float32)
    wo_r = inputs["wo_r"][hcols, :].astype(np.float32)
    wo_i = inputs["wo_i"][hcols, :].astype(np.float32)

    def stack_qk(wr, wi):
        # [C, NPAIR, 3, 128]: term banks (wr | wi | wr+wi), 2 heads packed in m
        out = np.empty((C, NPAIR, 3, D2), np.float32)
        for p in range(NPAIR):
            cs = slice(2 * p * D, (2 * p + 2) * D)  # both heads' 128 cols
            out[:, p, 0, :] = wr[:, cs]
            out[:, p, 1, :] = wi[:, cs]
            out[:, p, 2, :] = wr[:, cs] + wi[:, cs]
        return out.astype(np.float16)

    def stack_v(wr, wi):
        # [C, 3, HPC*64]
        out = np.empty((C, 3, HPC * D), np.float32)
        out[:, 0, :] = wr
        out[:, 1, :] = wi
        out[:, 2, :] = wr + wi
        return out.astype(np.float16)

    def stack_o(wr, wi):
        # [NPAIR, 3, 128, NEB, 512]: rows = (head a 64 | head b 64) d-rows
        out = np.empty((NPAIR, 3, D2, NEB, 512), np.float32)
        ws = wr + wi
        for p in range(NPAIR):
            rs = slice(2 * p * D, (2 * p + 2) * D)
            for eb in range(NEB):
                esl = slice(eb * 512, (eb + 1) * 512)
                out[p, 0, :, eb, :] = wr[rs, esl]
                out[p, 1, :, eb, :] = wi[rs, esl]
                out[p, 2, :, eb, :] = ws[rs, esl]
        return out.astype(np.float16)

    x_r = np.ascontiguousarray(inputs["inputs_real"][b]).astype(np.float32)
    x_i = np.ascontiguousarray(inputs["inputs_imag"][b]).astype(np.float32)
    c_r = np.ascontiguousarray(inputs["context_real"][b]).astype(np.float32)
    c_i = np.ascontiguousarray(inputs["context_imag"][b]).astype(np.float32)

    return {
        "x_r": x_r.astype(np.float16),
        "x_i": x_i.astype(np.float16),
        "x_s": (x_r + x_i).astype(np.float16),
        "c_r": c_r.astype(np.float16),
        "c_i": c_i.astype(np.float16),
        "c_s": (c_r + c_i).astype(np.float16),
        "wq": stack_qk(wq_r, wq_i),
        "wk": stack_qk(wk_r, wk_i),
        "wv": stack_v(wv_r, wv_i),
        "wo": stack_o(wo_r, wo_i),
    }


def get_program():
    if "nc" not in _CACHE:
        _CACHE["nc"] = _build_program()
    return _CACHE["nc"]


def kernel(**inputs):
    nc = get_program()
    in_maps = [_prep_core_inputs(inputs, core) for core in range(8)]
    res = run_bass_kernel_spmd(nc, in_maps, core_ids=list(range(8)))

    yr = np.zeros((B, L, C), np.float32)
    yi = np.zeros((B, L, C), np.float32)
    for core in range(8):
        b = core // 4
        yr[b] += res.results[core]["y_r"].astype(np.float32)
        yi[b] += res.results[core]["y_i"].astype(np.float32)
    yr += inputs["bo_r"][None, None, :]
    yi += inputs["bo_i"][None, None, :]
    return np.stack([yr, yi], axis=0)


# revision 20
# speedup vs baseline: 1.1212x; 1.1212x over previous
"""Trainium2 Bass kernel for nn_ComplexCrossAttention.

Sharding: 8 cores = 2 batches x 4 head-groups (4 heads each).
Each core computes, for its (b, head-group):
  - complex Q/K/V projections via Karatsuba (3 real matmuls instead of 4):
      t1 = xr@wr, t2 = xi@wi, t3 = (xr+xi)@(wr+wi)
      re = t1 - t2, im = t3 - t1 - t2
    Q/K pack two heads per matmul (m=128); the head-pair outputs are
    recombined into per-head (re|im) layout with two small SBUF->SBUF
    partition-shift DMAs per block.
  - attention scoresT = (qr.kr + qi.ki)*scale with s on partitions
  - softmax (no max-subtraction; scores are provably small) via exp + column-sum
  - av in transposed layout -> per-head OT [d2, l], normalized and packed
    into pair-major or/oi tiles (partition-shift DMAs for misaligned halves)
  - output projection via Karatsuba over pair-packed contraction blocks
Host sums the 4 partial y per batch and adds the bias.

Engine/queue roles (engines read at most one PSUM operand; DMA transposes
are only reliable from the sync queue):
  sync   - all DMA transposes + weight loads (in emission order), y stores
  scalar - Act: exp, PSUM->SBUF t2 copies
  vector - DVE: Karatsuba recombines (1 PSUM operand each), AV norm, recip
  gpsimd - Pool: xs/cs sums, softmax tree-sums, os=or+oi, shift DMAs (SWDGE)

Activations/weights are fp16 (full-rate on PE, 4x less rounding error than
bf16); x/ctx are pre-cast to fp16 on the host so transposes start
immediately. expT stays bf16 (exp can exceed fp16 max).
"""

import sys

import numpy as np

try:
    import concourse.bacc as bacc
except ImportError:  # pragma: no cover - fallback for bare environments
    sys.path.insert(0, "/opt/trn_rl_repo")
    import concourse.bacc as bacc

import concourse.mybir as mybir
import concourse.tile as tile
from concourse.bass_utils import run_bass_kernel_spmd

F32 = mybir.dt.float32
BF16 = mybir.dt.bfloat16
FP16 = mybir.dt.float16

# ---- problem constants (hardcoded per contract) ----
B, L, S, C = 2, 2048, 2048, 1024
H, D = 16, 64
SCALE = float(1.0 / np.sqrt(np.float32(D)))
HPC = 4          # heads per core
NPAIR = 2        # head pairs per core
D2 = 2 * D       # stacked (real|imag) head dim = 128
NCK = C // 128   # contraction chunks = 8
NLB = L // 512   # l-blocks = 4
NST = S // 128   # s-tiles = 16
NEB = 2          # e-blocks of 512 in C

ACT_DT = FP16    # activations + weights
EXP_DT = BF16    # expT (range: exp(scores) can exceed fp16 max)

_CACHE = {}


def _build_program():
    nc = bacc.Bacc("TRN2", target_bir_lowering=False, debug=False, num_devices=8)

    # per-core external inputs (host pre-cast to fp16)
    x_r = nc.dram_tensor("x_r", [L, C], ACT_DT, kind="ExternalInput")
    x_i = nc.dram_tensor("x_i", [L, C], ACT_DT, kind="ExternalInput")
    c_r = nc.dram_tensor("c_r", [S, C], ACT_DT, kind="ExternalInput")
    c_i = nc.dram_tensor("c_i", [S, C], ACT_DT, kind="ExternalInput")
    # Karatsuba weight banks (host-prepared):
    # wq/wk: [C, pair, term, 128] lhsT tiles (term: wr | wi | wr+wi; 2 heads packed)
    wq = nc.dram_tensor("wq", [C, NPAIR, 3, D2], ACT_DT, kind="ExternalInput")
    wk = nc.dram_tensor("wk", [C, NPAIR, 3, D2], ACT_DT, kind="ExternalInput")
    # wv: [C, term, HPC*64] rhs tiles
    wv = nc.dram_tensor("wv", [C, 3, HPC * D], ACT_DT, kind="ExternalInput")
    # wo: [pair, term, 128 (d-rows of head a|b), eb, 512] rhs tiles
    wo = nc.dram_tensor("wo", [NPAIR, 3, D2, NEB, 512], ACT_DT, kind="ExternalInput")

    y_r = nc.dram_tensor("y_r", [L, C], FP16, kind="ExternalOutput")
    y_i = nc.dram_tensor("y_i", [L, C], FP16, kind="ExternalOutput")

    import os
    dbg = None
    if os.environ.get("KARA_DEBUG"):
        dbg = {
            "dq": nc.dram_tensor("dq", [128, HPC, L], F32, kind="ExternalOutput"),
            "dk": nc.dram_tensor("dk", [128, HPC, S], F32, kind="ExternalOutput"),
            "dv": nc.dram_tensor("dv", [128, NST, HPC, 2, D], F32, kind="ExternalOutput"),
            "dor": nc.dram_tensor("dor", [128, NPAIR, 512], F32, kind="ExternalOutput"),
            "doi": nc.dram_tensor("doi", [128, NPAIR, 512], F32, kind="ExternalOutput"),
        }

    with tile.TileContext(nc) as tc:
        _emit(nc, tc, x_r, x_i, c_r, c_i, wq, wk, wv, wo, y_r, y_i, dbg)

    nc.compile()
    return nc


def _emit(nc, tc, x_r, x_i, c_r, c_i, wq, wk, wv, wo, y_r, y_i, dbg=None):
    from contextlib import ExitStack

    ctx = ExitStack()
    with ctx:
        attn_sb = ctx.enter_context(tc.tile_pool(name="attn_sb", bufs=1))

        # persistent attention operands
        qs = attn_sb.tile([128, HPC, L], ACT_DT)              # [d2, h, l]
        ks = attn_sb.tile([128, HPC, S], ACT_DT)              # [d2, h, s]
        vs = attn_sb.tile([128, NST, HPC, 2, D], ACT_DT)      # [s, st, h, ri, d]

        # ---------- P1: transpose-in x, Q projection (Karatsuba) ----------
        with (
            tc.tile_pool(name="xt", bufs=1) as xt_pool,
            tc.tile_pool(name="wq", bufs=1) as wq_pool,
            tc.tile_pool(name="qsc", bufs=2) as qsc_pool,
            tc.tile_pool(name="qus", bufs=2) as qus_pool,
            tc.tile_pool(name="ps_q", bufs=2, space="PSUM") as ps_q,
        ):
            wq_sb = wq_pool.tile([128, NCK, NPAIR, 3, D2], ACT_DT, tag="wq")
            nc.sync.dma_start(
                out=wq_sb, in_=wq.rearrange("(ck p) pr t m -> p ck pr t m", p=128)
            )
            xt = [xt_pool.tile([128, NCK, L], ACT_DT, tag=f"xt{t}", name=f"xt{t}")
                  for t in range(3)]
            for ck in range(NCK):
                for t, src in enumerate((x_r, x_i)):
                    nc.sync.dma_start(
                        out=xt[t][:, ck, :],
                        in_=src[:, ck * 128:(ck + 1) * 128],
                        transpose=True,
                    )
                # xs = xr + xi on the (idle) Pool engine, per chunk
                nc.gpsimd.tensor_add(
                    out=xt[2][:, ck, :], in0=xt[0][:, ck, :], in1=xt[1][:, ck, :])
            for p in range(NPAIR):
                a, b = 2 * p, 2 * p + 1
                for lb in range(NLB):
                    lsl = slice(lb * 512, (lb + 1) * 512)
                    pt = ps_q.tile([128, 3, 512], F32, tag="pq", name="pq")
                    for ck in range(NCK):
                        for t in range(3):
                            nc.tensor.matmul(
                                pt[:, t, :],
                                wq_sb[:, ck, p, t, :],
                                xt[t][:, ck, lsl],
                                start=(ck == 0),
                                stop=(ck == NCK - 1),
                                skip_group_check=True,
                            )
                    # re = t1-t2, im = t3-t1-t2 ; halves for head a (rows 0:64)
                    # and head b (rows 64:128); misaligned halves go via a
                    # scratch tile + partition-shift DMA. Engines read at most
                    # one PSUM operand, so t2 and u=t1+t2 stage via SBUF.
                    qsc = qsc_pool.tile([128, 512], ACT_DT, tag="qsc")
                    t2s = qus_pool.tile([128, 512], F32, tag="t2s")
                    us = qus_pool.tile([128, 512], F32, tag="us")
                    nc.scalar.copy(out=t2s, in_=pt[:, 1, :])
                    nc.vector.tensor_sub(
                        out=qs[0:64, a, lsl], in0=pt[0:64, 0, :], in1=t2s[0:64, :])
                    nc.vector.tensor_sub(
                        out=qsc[64:128, :], in0=pt[64:128, 0, :], in1=t2s[64:128, :])
                    nc.vector.tensor_add(
                        out=us, in0=pt[:, 0, :], in1=t2s)
                    nc.vector.tensor_sub(
                        out=qsc[0:64, :], in0=pt[0:64, 2, :], in1=us[0:64, :])
                    nc.vector.tensor_sub(
                        out=qs[64:128, b, lsl], in0=pt[64:128, 2, :], in1=us[64:128, :])
                    nc.gpsimd.dma_start(out=qs[0:64, b, lsl], in_=qsc[64:128, :])
                    nc.gpsimd.dma_start(out=qs[64:128, a, lsl], in_=qsc[0:64, :])

        # ---------- P3: transpose ctx, K and V projections (Karatsuba) ----
        # score/exp pools open across P3 so the scheduler can hoist
        # scoresT+exp of early heads into K/V-phase gaps.
        exp_pool = ctx.enter_context(tc.tile_pool(name="exp", bufs=1))
        ps_s = ctx.enter_context(tc.tile_pool(name="ps_s", bufs=1, space="PSUM"))

        with (
            tc.tile_pool(name="ct", bufs=1) as ct_pool,
            tc.tile_pool(name="wkv", bufs=1) as wkv_pool,
            tc.tile_pool(name="ksc", bufs=2) as ksc_pool,
            tc.tile_pool(name="kus", bufs=2) as kus_pool,
            tc.tile_pool(name="ps_kv", bufs=2, space="PSUM") as ps_kv,
        ):
            wk_sb = wkv_pool.tile([128, NCK, NPAIR, 3, D2], ACT_DT, tag="wk")
            nc.sync.dma_start(
                out=wk_sb, in_=wk.rearrange("(ck p) pr t m -> p ck pr t m", p=128)
            )
            wv_sb = wkv_pool.tile([128, NCK, 3, HPC * D], ACT_DT, tag="wv")
            nc.sync.dma_start(
                out=wv_sb, in_=wv.rearrange("(ck p) t n -> p ck t n", p=128)
            )
            ct = [ct_pool.tile([128, NCK, S], ACT_DT, tag=f"ct{t}", name=f"ct{t}")
                  for t in range(3)]
            for ck in range(NCK):
                for t, src in enumerate((c_r, c_i)):
                    nc.sync.dma_start(
                        out=ct[t][:, ck, :],
                        in_=src[:, ck * 128:(ck + 1) * 128],
                        transpose=True,
                    )
                nc.gpsimd.tensor_add(
                    out=ct[2][:, ck, :], in0=ct[0][:, ck, :], in1=ct[1][:, ck, :])

            def k_iter(p, sb):
                a, b = 2 * p, 2 * p + 1
                ssl = slice(sb * 512, (sb + 1) * 512)
                pk = ps_kv.tile([128, 3, 512], F32, tag="pkv", name="pk")
                for ck in range(NCK):
                    for t in range(3):
                        nc.tensor.matmul(
                            pk[:, t, :],
                            wk_sb[:, ck, p, t, :],
                            ct[t][:, ck, ssl],
                            start=(ck == 0),
                            stop=(ck == NCK - 1),
                            skip_group_check=True,
                        )
                ksc = ksc_pool.tile([128, 512], ACT_DT, tag="ksc")
                t2k = kus_pool.tile([128, 512], F32, tag="t2k")
                uk = kus_pool.tile([128, 512], F32, tag="uk")
                nc.scalar.copy(out=t2k, in_=pk[:, 1, :])
                nc.vector.tensor_sub(
                    out=ks[0:64, a, ssl], in0=pk[0:64, 0, :], in1=t2k[0:64, :])
                nc.vector.tensor_sub(
                    out=ksc[64:128, :], in0=pk[64:128, 0, :], in1=t2k[64:128, :])
                nc.vector.tensor_add(out=uk, in0=pk[:, 0, :], in1=t2k)
                nc.vector.tensor_sub(
                    out=ksc[0:64, :], in0=pk[0:64, 2, :], in1=uk[0:64, :])
                nc.vector.tensor_sub(
                    out=ks[64:128, b, ssl], in0=pk[64:128, 2, :], in1=uk[64:128, :])
                nc.gpsimd.dma_start(out=ks[0:64, b, ssl], in_=ksc[64:128, :])
                nc.gpsimd.dma_start(out=ks[64:128, a, ssl], in_=ksc[0:64, :])

            def v_iter(st):
                pv = ps_kv.tile([128, 3, 512], F32, tag="pkv", name="pv")
                for ck in range(NCK):
                    for t in range(3):
                        nc.tensor.matmul(
                            pv[:, t, 0:HPC * D],
                            ct[t][:, ck, st * 128:(st + 1) * 128],
                            wv_sb[:, ck, t, :],
                            start=(ck == 0),
                            stop=(ck == NCK - 1),
                            skip_group_check=True,
                        )
                t2v = kus_pool.tile([128, HPC * D], F32, tag="t2v")
                uv = kus_pool.tile([128, HPC * D], F32, tag="uv")
                nc.scalar.copy(out=t2v, in_=pv[:, 1, 0:HPC * D])
                nc.vector.tensor_sub(
                    out=vs[:, st, :, 0, :], in0=pv[:, 0, 0:HPC * D], in1=t2v)
                nc.vector.tensor_add(out=uv, in0=pv[:, 0, 0:HPC * D], in1=t2v)
                nc.vector.tensor_sub(
                    out=vs[:, st, :, 1, :], in0=pv[:, 2, 0:HPC * D], in1=uv)

            # interleave K and V iterations (they share the rotating psum pool)
            for i in range(8):
                k_iter(i // 4, i % 4)
                v_iter(2 * i)
                v_iter(2 * i + 1)

        if dbg is not None:
            nc.gpsimd.dma_start(out=dbg["dq"][:, :, :], in_=qs)
            nc.gpsimd.dma_start(out=dbg["dk"][:, :, :], in_=ks)
            nc.gpsimd.dma_start(out=dbg["dv"][:, :, :, :, :], in_=vs)

        # ---------- P4+P5: attention + output projection, software-pipelined ----
        with (
            tc.tile_pool(name="late", bufs=1) as late_pool,
            tc.tile_pool(name="exp2", bufs=3) as exp_pool2,
            tc.tile_pool(name="otp", bufs=2) as ot_pool,
            tc.tile_pool(name="osc", bufs=2) as osc_pool,
            tc.tile_pool(name="ysb", bufs=2) as ysb_pool,
            tc.tile_pool(name="ps_d", bufs=1, space="PSUM") as ps_d,
            tc.tile_pool(name="ps_o", bufs=1, space="PSUM") as ps_o,
            tc.tile_pool(name="ps_y", bufs=1, space="PSUM") as ps_y,
        ):
            ones = late_pool.tile([128, D2], EXP_DT)
            nc.vector.memset(ones, 1.0)
            wo_sb = late_pool.tile([128, NPAIR, 3, NEB, 512], ACT_DT, tag="wo")
            nc.sync.dma_start(out=wo_sb, in_=wo.rearrange("pr t p eb e -> p pr t eb e"))

            otiles = {}

            def attn_head(lb, h):
                lsl = slice(lb * 512, (lb + 1) * 512)
                p, is_b = h // 2, h % 2
                orp, oip, osp = otiles[lb]
                pool_h = exp_pool if lb == 0 else exp_pool2
                expt = pool_h.tile([128, NST, 512], EXP_DT, tag="expt", name="expt")
                for pr in range(NST // 2):
                    pscore = ps_s.tile([128, 2, 512], F32, tag="pscore")
                    for j in range(2):
                        st = 2 * pr + j
                        nc.tensor.matmul(
                            pscore[:, j, :],
                            ks[:, h, st * 128:(st + 1) * 128],
                            qs[:, h, lsl],
                            start=True,
                            stop=True,
                            skip_group_check=True,
                        )
                    nc.scalar.activation(
                        out=expt[:, 2 * pr:2 * pr + 2, :],
                        in_=pscore,
                        func=mybir.ActivationFunctionType.Exp,
                        scale=SCALE,
                    )
                # av: OT[d2, l] accumulated over s-tiles (reads expt first)
                pav = ps_o.tile([128, 512], F32, tag="pav")
                for st in range(NST):
                    nc.tensor.matmul(
                        pav,
                        vs[:, st, h, :, :],
                        expt[:, st, :],
                        start=(st == 0),
                        stop=(st == NST - 1),
                    )
                # in-place pairwise tree-sum of the 16 s-tiles, all on Pool
                for step in (1, 2, 4, 8):
                    for j in range(0, NST, 2 * step):
                        nc.gpsimd.tensor_add(
                            out=expt[:, j, :], in0=expt[:, j, :],
                            in1=expt[:, j + step, :])
                pden = ps_d.tile([128, 512], F32, tag="pden")
                nc.tensor.matmul(pden, ones, expt[:, 0, :], start=True, stop=True)
                recip = ot_pool.tile([128, 512], F32, tag="recip")
                nc.vector.reciprocal(out=recip, in_=pden)
                # normalize + pack into pair-major tiles; misaligned halves
                # go via scratch + partition-shift DMA
                osc = osc_pool.tile([128, 512], ACT_DT, tag="osc")
                if is_b == 0:  # head a: or aligned, oi shifts down
                    nc.vector.tensor_mul(
                        out=orp[0:64, p, :], in0=pav[0:64, :], in1=recip[0:64, :])
                    nc.vector.tensor_mul(
                        out=osc[64:128, :], in0=pav[64:128, :], in1=recip[64:128, :])
                    nc.gpsimd.dma_start(out=oip[0:64, p, :], in_=osc[64:128, :])
                else:          # head b: oi aligned, or shifts up
                    nc.vector.tensor_mul(
                        out=osc[0:64, :], in0=pav[0:64, :], in1=recip[0:64, :])
                    nc.vector.tensor_mul(
                        out=oip[64:128, p, :], in0=pav[64:128, :], in1=recip[64:128, :])
                    nc.gpsimd.dma_start(out=orp[64:128, p, :], in_=osc[0:64, :])

            def oproj_piece(lb, jt):
                lt = lb * 4 + jt
                lrow = slice(lt * 128, (lt + 1) * 128)
                jsl = slice(jt * 128, (jt + 1) * 128)
                orp, oip, osp = otiles[lb]
                for eb in range(NEB):
                    esl = slice(eb * 512, (eb + 1) * 512)
                    py = ps_y.tile([128, 3, 512], F32, tag="py", name="py")
                    for t, term in enumerate((orp, oip, osp)):
                        for p in range(NPAIR):
                            nc.tensor.matmul(
                                py[:, t, :],
                                term[:, p, jsl],
                                wo_sb[:, p, t, eb, :],
                                start=(p == 0),
                                stop=(p == NPAIR - 1),
                                skip_group_check=True,
                            )
                    t2o = ysb_pool.tile([128, 512], F32, tag="t2o")
                    uo = ysb_pool.tile([128, 512], F32, tag="uo")
                    nc.scalar.copy(out=t2o, in_=py[:, 1, :])
                    yr_t = ysb_pool.tile([128, 512], FP16, tag="yrt")
                    nc.vector.tensor_sub(out=yr_t, in0=py[:, 0, :], in1=t2o)
                    nc.sync.dma_start(out=y_r[lrow, esl], in_=yr_t)
                    nc.vector.tensor_add(out=uo, in0=py[:, 0, :], in1=t2o)
                    yi_t = ysb_pool.tile([128, 512], FP16, tag="yit")
                    nc.vector.tensor_sub(out=yi_t, in0=py[:, 2, :], in1=uo)
                    nc.sync.dma_start(out=y_i[lrow, esl], in_=yi_t)

            for lb in range(NLB):
                otiles[lb] = (
                    ot_pool.tile([128, NPAIR, 512], ACT_DT, tag="orp", name="orp"),
                    ot_pool.tile([128, NPAIR, 512], ACT_DT, tag="oip", name="oip"),
                    ot_pool.tile([128, NPAIR, 512], ACT_DT, tag="osp", name="osp"),
                )
                for h in range(HPC):
                    attn_head(lb, h)
                    # software-pipeline: emit previous block's O-projection
                    # between this block's heads so PE stalls interleave
                    if lb > 0:
                        oproj_piece(lb - 1, h)
                orp, oip, osp = otiles[lb]
                nc.gpsimd.tensor_add(out=osp, in0=orp, in1=oip)
                if dbg is not None and lb == 0:
                    nc.gpsimd.dma_start(out=dbg["dor"][:, :, :], in_=orp)
                    nc.gpsimd.dma_start(out=dbg["doi"][:, :, :], in_=oip)
            for jt in range(4):
                oproj_piece(NLB - 1, jt)


def _prep_core_inputs(inputs, core):
    """Slice + host-prepare activation/weight layouts for one core."""
    b = core // 4
    g = core % 4
    hcols = slice(g * HPC * D, (g + 1) * HPC * D)  # 256 channel cols/rows

    wq_r = inputs["wq_r"][:, hcols].astype(np.float32)
    wq_i = inputs["wq_i"][:, hcols].astype(np.float32)
    wk_r = inputs["wk_r"][:, hcols].astype(np.float32)
    wk_i = inputs["wk_i"][:, hcols].astype(np.float32)
    wv_r = inputs["wv_r"][:, hcols].astype(np.float32)
    wv_i = inputs["wv_i"][:, hcols].astype(np.float32)
    wo_r = inputs["wo_r"][hcols, :].astype(np.float32)
    wo_i = inputs["wo_i"][hcols, :].astype(np.float32)

    def stack_qk(wr, wi):
        # [C, NPAIR, 3, 128]: term banks (wr | wi | wr+wi), 2 heads packed in m
        out = np.empty((C, NPAIR, 3, D2), np.float32)
        for p in range(NPAIR):
            cs = slice(2 * p * D, (2 * p + 2) * D)  # both heads' 128 cols
            out[:, p, 0, :] = wr[:, cs]
            out[:, p, 1, :] = wi[:, cs]
            out[:, p, 2, :] = wr[:, cs] + wi[:, cs]
        return out.astype(np.float16)

    def stack_v(wr, wi):
        # [C, 3, HPC*64]
        out = np.empty((C, 3, HPC * D), np.float32)
        out[:, 0, :] = wr
        out[:, 1, :] = wi
        out[:, 2, :] = wr + wi
        return out.astype(np.float16)

    def stack_o(wr, wi):
        # [NPAIR, 3, 128, NEB, 512]: rows = (head a 64 | head b 64) d-rows
        out = np.empty((NPAIR, 3, D2, NEB, 512), np.float32)
        ws = wr + wi
        for p in range(NPAIR):
            rs = slice(2 * p * D, (2 * p + 2) * D)
            for eb in range(NEB):
                esl = slice(eb * 512, (eb + 1) * 512)
                out[p, 0, :, eb, :] = wr[rs, esl]
                out[p, 1, :, eb, :] = wi[rs, esl]
                out[p, 2, :, eb, :] = ws[rs, esl]
        return out.astype(np.float16)

    return {
        "x_r": np.ascontiguousarray(inputs["inputs_real"][b]).astype(np.float16),
        "x_i": np.ascontiguousarray(inputs["inputs_imag"][b]).astype(np.float16),
        "c_r": np.ascontiguousarray(inputs["context_real"][b]).astype(np.float16),
        "c_i": np.ascontiguousarray(inputs["context_imag"][b]).astype(np.float16),
        "wq": stack_qk(wq_r, wq_i),
        "wk": stack_qk(wk_r, wk_i),
        "wv": stack_v(wv_r, wv_i),
        "wo": stack_o(wo_r, wo_i),
    }


def get_program():
    if "nc" not in _CACHE:
        _CACHE["nc"] = _build_program()
    return _CACHE["nc"]


def kernel(**inputs):
    nc = get_program()
    in_maps = [_prep_core_inputs(inputs, core) for core in range(8)]
    res = run_bass_kernel_spmd(nc, in_maps, core_ids=list(range(8)))

    yr = np.zeros((B, L, C), np.float32)
    yi = np.zeros((B, L, C), np.float32)
    for core in range(8):
        b = core // 4
        yr[b] += res.results[core]["y_r"].astype(np.float32)
        yi[b] += res.results[core]["y_i"].astype(np.float32)
    yr += inputs["bo_r"][None, None, :]
    yi += inputs["bo_i"][None, None, :]
    return np.stack([yr, yi], axis=0)
